# revision 4
# baseline (speedup 1.0000x reference)
"""Multi-head attention forward (B=8, S=1024, H=16, D=64) on 8 TRN2 NeuronCores.

Sharding: pure data-parallel over batch - core b computes batch element b
end-to-end (QKV projections + 16-head attention). Zero collectives.

v2 design (vs the PE-transpose-heavy baseline):
  - ZERO PE transposes. x^T comes from XBAR DMA-transposes of bf16 staging
    chunks (cast f32->bf16 on the SWDGE load). V is computed directly in
    [seq, dim] layout (lhsT = x_to^T slice, rhs = Wv), so the ctx matmul's
    stationary V' strips come straight out of the projection. The final
    ctx^T -> [seq, dim] flip is another XBAR DMA-transpose.
  - The pair loop is software-pipelined at (jt, ih) unit granularity:
    each unit issues the scores matmuls for pair p, the ctx matmuls for
    pair p-1, and filler projection matmuls for pair p+1/V, so the ScalarE
    exp stream always overlaps dense PE work and the single scores PSUM
    tile never stalls the PE.
  - Input DMAs are ordered so the PE starts projecting at ~3us: Wq pair-0
    slice first, then x_from chunks (projection matmuls chase the chunks),
    then Wk slice 0 + x_to, then Wv in two column halves, then the
    remaining per-pair Wq/Wk slices.
  - Softmax normalization: the ones-column denominator row of each ctx
    PSUM tile is reciprocal'd on DVE, partition-broadcast on GpSimd, and
    multiplied in during the PSUM->SBUF drain (no extra pass).
"""

import numpy as np
from contextlib import ExitStack

import concourse.bass as bass
import concourse.mybir as mybir
import concourse.tile as tile
from concourse import bacc
from concourse.bass_utils import run_bass_kernel_spmd

B, S, H, D = 8, 1024, 16, 64
W = H * D  # 1024
P = 128
N_CORES = 8
F32 = mybir.dt.float32
BF16 = mybir.dt.bfloat16
AF = mybir.ActivationFunctionType
ALU = mybir.AluOpType

ST = S // P   # 8 s-tiles
KT_ = W // P  # 8 contraction tiles
IH = 2        # 512-wide halves of the moving dim
HD1 = D + 1   # 65: V' width per head (64 V cols + ones col)
NP = H // 2   # 8 head pairs


def _dedup_ldweights(nc):
    """Drop InstLdweights that reload the exact weights already resident in
    the PE array (kt-outer projection loops share one stationary between the
    two ih-half matmuls). Runs post-compile, so syncs are final: only
    duplicates with empty sync_info, separated from the previous load purely
    by matmuls on the PE stream, are removed."""
    removed = 0
    for f in nc.m.functions:
        for blk in f.blocks:
            ins = blk.instructions
            last_key = None
            to_remove = []
            for i in ins:
                if str(getattr(i, "engine", None)) != "EngineType.PE":
                    continue
                tn = type(i).__name__
                if tn == "InstLdweights":
                    si = i.sync_info
                    clean = si is None or (not si.on_wait and not si.on_update)
                    key = (str(i.ins), str(getattr(i, "is_transpose", None)),
                           str(getattr(i, "tile_position", None)),
                           str(getattr(i, "perf_mode", None)))
                    if clean and key == last_key:
                        to_remove.append(i)
                    else:
                        last_key = key
                elif tn != "InstMatmult":
                    last_key = None
            for i in to_remove:
                ins.remove(i)
            removed += len(to_remove)
    return removed


def build_kernel():
    nc = bacc.Bacc(trn_type="TRN2", target_bir_lowering=False, debug=False,
                   num_devices=N_CORES)

    xf_ext = nc.dram_tensor("from_tensor", [S, W], F32, kind="ExternalInput").ap()
    xt_ext = nc.dram_tensor("to_tensor", [S, W], F32, kind="ExternalInput").ap()
    wq_ext = nc.dram_tensor("Wq", [W, W], F32, kind="ExternalInput").ap()
    bq_ext = nc.dram_tensor("bq", [W], F32, kind="ExternalInput").ap()
    wk_ext = nc.dram_tensor("Wk", [W, W], F32, kind="ExternalInput").ap()
    bk_ext = nc.dram_tensor("bk", [W], F32, kind="ExternalInput").ap()
    wv_ext = nc.dram_tensor("Wv", [W, W], F32, kind="ExternalInput").ap()
    bv_ext = nc.dram_tensor("bv", [W], F32, kind="ExternalInput").ap()
    out_ext = nc.dram_tensor("out", [S, W], F32, kind="ExternalOutput").ap()

    with tile.TileContext(nc) as tc, ExitStack() as top:
        const = top.enter_context(tc.tile_pool(name="const", bufs=1))
        big = top.enter_context(tc.tile_pool(name="big", bufs=1))

        # per-pair per-partition bias scalars for Q^T/K^T (m on partitions)
        bq_sb = const.tile([P, KT_], F32, tag="bq")
        bk_sb = const.tile([P, KT_], F32, tag="bk")
        bv_row = const.tile([1, W], F32, tag="bvr")
        bvb = const.tile([P, W], F32, tag="bvb")

        # xT_all[p, kt*S + s] = x[s, kt*128+p]   (bf16)
        xTf_all = big.tile([P, KT_ * S], BF16, tag="xTf")
        xTt_all = big.tile([P, KT_ * S], BF16, tag="xTt")
        # w_all[p, kt*W + f] = Wx[kt*128+p, f]   (bf16, cast on SWDGE)
        wq_all = big.tile([P, KT_ * W], BF16, tag="wq")
        wk_all = big.tile([P, KT_ * W], BF16, tag="wk")
        wv_all = big.tile([P, KT_ * W], BF16, tag="wv")
        # V in [j, m] layout with interleaved ones columns:
        # V_sb[j, st*16*HD1 + h*HD1 + d], col h*HD1+64 == 1.0
        V_sb = big.tile([P, ST * H * HD1], BF16, tag="vsb")

        def load_w_cols(dst_all, src, c0, c1):
            """Weight columns [c0, c1), all 8 kt chunks, cast to bf16."""
            nc.gpsimd.dma_start(
                dst_all.rearrange("p (t f) -> p t f", f=W)[:, :, c0:c1],
                src.rearrange("(t p) f -> p t f", p=P)[:, :, c0:c1])

        # ---- DMA schedule (gpsimd/SWDGE queue, casts f32->bf16 in flight) ----
        nc.gpsimd.dma_start(bq_sb[:], bq_ext.rearrange("(t p) -> p t", p=P))
        nc.gpsimd.dma_start(bk_sb[:], bk_ext.rearrange("(t p) -> p t", p=P))
        nc.gpsimd.dma_start(bv_row[:], bv_ext.rearrange("(p w) -> p w", p=1))
        nc.gpsimd.partition_broadcast(bvb[:], bv_row[:])

        with ExitStack() as stg_ctx:
            stg = stg_ctx.enter_context(tc.tile_pool(name="stg", bufs=3))

            def stage_x(x_ext, xT_all):
                """Load one x chunk-wise (cast to bf16) + XBAR-transpose it."""
                for ch in range(ST):
                    xs = stg.tile([P, W], BF16, tag="xs", name=f"xs{ch}")
                    nc.gpsimd.dma_start(
                        xs[:], x_ext.rearrange("(t p) w -> p t w", p=P)[:, ch, :])
                    # out[wp, wt, s] = xs[s, wt*128+wp]
                    nc.sync.dma_start(
                        xT_all.rearrange("p (t s) -> p t s", s=S)[
                            :, :, ch * P:(ch + 1) * P],
                        xs[:], transpose=True)

            load_w_cols(wq_all, wq_ext, 0, P)          # pair 0 Q cols
            stage_x(xf_ext, xTf_all)
            load_w_cols(wk_all, wk_ext, 0, P)          # pair 0 K cols
            stage_x(xt_ext, xTt_all)
            load_w_cols(wv_all, wv_ext, 0, 512)        # V half A (heads 0-7)
            load_w_cols(wq_all, wq_ext, P, 2 * P)      # pair 1
            load_w_cols(wk_all, wk_ext, P, 2 * P)
            load_w_cols(wq_all, wq_ext, 2 * P, 3 * P)  # pair 2
            load_w_cols(wk_all, wk_ext, 2 * P, 3 * P)
            load_w_cols(wv_all, wv_ext, 512, 1024)     # V half B (heads 8-15)
            load_w_cols(wq_all, wq_ext, 3 * P, W)      # pairs 3-7
            load_w_cols(wk_all, wk_ext, 3 * P, W)

            # ---- pair loop ----
            with ExitStack() as ph2:
                pp_pool = ph2.enter_context(tc.tile_pool(name="pp", bufs=1))
                et_pool = ph2.enter_context(tc.tile_pool(name="et", bufs=20))
                sm_pool = ph2.enter_context(tc.tile_pool(name="sm", bufs=1))
                ps_proj = ph2.enter_context(
                    tc.tile_pool(name="ps_proj", bufs=2, space="PSUM"))
                ps_s = ph2.enter_context(
                    tc.tile_pool(name="ps_s", bufs=1, space="PSUM"))
                ps_c = ph2.enter_context(
                    tc.tile_pool(name="ps_c", bufs=2, space="PSUM"))

                def gen_qk_proj(QTp, KTp, mt):
                    """Q^T/K^T projection for pair mt, kt-outer (the two
                    ih-half matmuls share one ldweights after dedup)."""
                    for (dstT, w_all, x_all, b_sb) in (
                            (QTp, wq_all, xTf_all, bq_sb),
                            (KTp, wk_all, xTt_all, bk_sb)):
                        ps = {}
                        for ih in range(IH):
                            ps[ih] = ps_proj.tile([P, 512], F32, tag="proj",
                                                  name="pp")
                        for kt in range(KT_):
                            for ih in range(IH):
                                nc.tensor.matmul(
                                    ps[ih][:],
                                    lhsT=w_all[:, kt * W + mt * P:
                                               kt * W + mt * P + P],
                                    rhs=x_all[:, kt * S + ih * 512:
                                              kt * S + (ih + 1) * 512],
                                    start=(kt == 0), stop=(kt == KT_ - 1))
                            if kt % 4 == 3:
                                yield
                        for ih in range(IH):
                            nc.vector.tensor_scalar_add(
                                dstT[:, ih * 512:(ih + 1) * 512], ps[ih][:],
                                b_sb[:, mt:mt + 1])
                        yield

                def gen_v_proj(half):
                    """V projection for one 512-wide column half (8 heads),
                    all 8 s-tiles; yields after each s-tile."""
                    for st in range(ST):
                        pv = ps_proj.tile([P, 512], F32, tag="proj", name="pv")
                        for kt in range(KT_):
                            nc.tensor.matmul(
                                pv[:],
                                lhsT=xTt_all[:, kt * S + st * P:
                                             kt * S + (st + 1) * P],
                                rhs=wv_all[:, kt * W + half * 512:
                                           kt * W + (half + 1) * 512],
                                start=(kt == 0), stop=(kt == KT_ - 1))
                        base = st * H * HD1 + half * 8 * HD1
                        dst = V_sb[:, base: base + 8 * HD1].rearrange(
                            "p (h c) -> p h c", c=HD1)
                        nc.vector.tensor_tensor(
                            dst[:, :, 0:D],
                            pv[:].rearrange("p (h c) -> p h c", c=D),
                            bvb[:, half * 512:(half + 1) * 512].rearrange(
                                "p (h c) -> p h c", c=D),
                            ALU.add)
                        nc.vector.memset(dst[:, :, D:HD1], 1.0)
                        yield

                filler = []      # FIFO of generators of PE work chunks
                done_gens = set()

                def pull_filler():
                    while filler:
                        try:
                            next(filler[0])
                            return
                        except StopIteration:
                            done_gens.add(id(filler[0]))
                            filler.pop(0)

                def drain_gen(g):
                    """Run generator g to completion (out-of-band safety:
                    all its PE work must precede dependents in the PE queue,
                    and emission order IS queue order)."""
                    if g is None or id(g) in done_gens:
                        return
                    while True:
                        try:
                            next(g)
                        except StopIteration:
                            done_gens.add(id(g))
                            if g in filler:
                                filler.remove(g)
                            return

                QK = {}      # pair -> (QTp, KTp)
                qk_gen = {}  # pair -> generator
                v_gen = {}   # half -> generator

                def emit_pair_qk(p):
                    QTp = pp_pool.tile([P, S], BF16, tag="qt", bufs=2,
                                       name="QTp")
                    KTp = pp_pool.tile([P, S], BF16, tag="kt", bufs=2,
                                       name="KTp")
                    QK[p] = (QTp, KTp)
                    g = gen_qk_proj(QTp, KTp, p)
                    qk_gen[p] = g
                    filler.append(g)

                Et = {}  # (pair, jt, ih) -> exp tile
                pc = {}  # (pair, hh) -> ctx psum tile

                def emit_scores_unit(p, jt, ih):
                    QTp, KTp = QK[p]
                    pss = ps_s.tile([P, 1024], F32, tag="pss", name="pss")
                    for hh in range(2):
                        ho = hh * D
                        nc.tensor.matmul(
                            pss[:, hh * 512:(hh + 1) * 512],
                            lhsT=KTp[ho:ho + D, jt * P: jt * P + P],
                            rhs=QTp[ho:ho + D, ih * 512:(ih + 1) * 512],
                            start=True, stop=True)
                    et = et_pool.tile([P, 1024], BF16, tag="et", name="et")
                    nc.scalar.activation(et[:], pss[:], AF.Exp, scale=0.125)
                    Et[(p, jt, ih)] = et

                def emit_ctx_unit(p, jt, ih):
                    if jt == 0 and ih == 0:
                        for hh in range(2):
                            pc[(p, hh)] = ps_c.tile([HD1, S], F32, tag="pcc",
                                                    name="pcc")
                    et = Et.pop((p, jt, ih))
                    for hh in range(2):
                        h = p * 2 + hh
                        nc.tensor.matmul(
                            pc[(p, hh)][:, ih * 512:(ih + 1) * 512],
                            lhsT=V_sb[:, (jt * H + h) * HD1:
                                      (jt * H + h) * HD1 + HD1],
                            rhs=et[:, hh * 512:(hh + 1) * 512],
                            start=(jt == 0), stop=(jt == ST - 1))

                def emit_out(p):
                    """Normalize + XBAR-transpose + DMA out for pair p."""
                    out_p = sm_pool.tile([P, ST * P], BF16, tag="outp", bufs=2,
                                         name="out_p")
                    for hh in range(2):
                        pch = pc.pop((p, hh))
                        rrow = sm_pool.tile([1, S], F32, tag="rrow", bufs=2,
                                            name="rrow")
                        nc.vector.reciprocal(rrow[:], pch[D:HD1, :])
                        rb = sm_pool.tile([D, S], F32, tag="rb", bufs=2,
                                          name="rb")
                        nc.gpsimd.partition_broadcast(rb[:], rrow[:])
                        ctxn = sm_pool.tile([D, S], BF16, tag="ctxn", bufs=3,
                                            name="ctxn")
                        nc.vector.tensor_tensor(
                            ctxn[:], pch[0:D, :], rb[:], ALU.mult)
                        # out_p[s, it*128 + hh*64 + d] = ctxn[d, it*128+s]
                        nc.sync.dma_start(
                            out_p.rearrange("p (t c) -> p t c", c=P)[
                                :, :, hh * D:(hh + 1) * D],
                            ctxn[:], transpose=True)
                    nc.gpsimd.dma_start(
                        out_ext.rearrange("(t p) (g c) -> p t g c", p=P, c=P)[
                            :, :, p, :],
                        out_p.rearrange("p (t c) -> p t c", c=P))

                # ---- pipeline ----
                emit_pair_qk(0)
                drain_gen(qk_gen[0])  # pair 0's Q/K must fully precede units
                emit_pair_qk(1)
                v_gen[0] = gen_v_proj(0)
                filler.append(v_gen[0])

                for p in range(NP):
                    if p == 1:
                        v_gen[1] = gen_v_proj(1)
                        filler.append(v_gen[1])
                    if p + 2 < NP:
                        emit_pair_qk(p + 2)
                    # safety: everything pair p's scores / pair p-1's ctx
                    # read must already be in the PE queue
                    drain_gen(qk_gen.get(p))
                    if p > 0:
                        drain_gen(v_gen.get((p - 1) // 4))
                    for jt in range(ST):
                        for ih in range(IH):
                            emit_scores_unit(p, jt, ih)
                            if p > 0:
                                emit_ctx_unit(p - 1, jt, ih)
                            pull_filler()
                            if p == 0:
                                pull_filler()
                    if p > 0:
                        emit_out(p - 1)
                # drain pair NP-1's ctx + out
                for jt in range(ST):
                    for ih in range(IH):
                        emit_ctx_unit(NP - 1, jt, ih)
                        pull_filler()
                emit_out(NP - 1)

    nc.compile()
    _dedup_ldweights(nc)
    return nc


def run(inputs, trace=False, trace_kwargs=None):
    """inputs: dict of full-shape np arrays as in reference.setup_inputs()."""
    nc = build_kernel()
    in_maps = []
    for b in range(N_CORES):
        in_maps.append({
            "from_tensor": np.ascontiguousarray(np.asarray(inputs["from_tensor"][b], dtype=np.float32)),
            "to_tensor": np.ascontiguousarray(np.asarray(inputs["to_tensor"][b], dtype=np.float32)),
            "Wq": np.asarray(inputs["Wq"], dtype=np.float32),
            "bq": np.asarray(inputs["bq"], dtype=np.float32),
            "Wk": np.asarray(inputs["Wk"], dtype=np.float32),
            "bk": np.asarray(inputs["bk"], dtype=np.float32),
            "Wv": np.asarray(inputs["Wv"], dtype=np.float32),
            "bv": np.asarray(inputs["bv"], dtype=np.float32),
        })
    res = run_bass_kernel_spmd(nc, in_maps, core_ids=list(range(N_CORES)),
                               trace=trace, **(trace_kwargs or {}))
    out = np.stack([np.asarray(res.results[b]["out"]) for b in range(N_CORES)],
                   axis=0).astype(np.float32)
    return out, res


def kernel(**inputs):
    out, _ = run(inputs, trace=False)
    return out


# revision 6
# speedup vs baseline: 1.3506x; 1.3506x over previous
"""Multi-head attention forward (B=8, S=1024, H=16, D=64) on 8 TRN2 NeuronCores.

Sharding: pure data-parallel over batch - core b computes batch element b
end-to-end (QKV projections + 16-head attention). Zero collectives.

v3 design notes:
  - ZERO PE transposes. x^T comes from XBAR DMA-transposes of bf16 staging
    chunks (cast f32->bf16 on the SWDGE load). V is computed directly in
    [seq, dim] layout, and the final ctx^T -> [seq, dim] flip is another
    XBAR DMA-transpose.
  - Tile dependencies are whole-tile, so every independently-DMA'd slice
    gets its own tile: per-pair Wq/Wk tiles, per-half Wv / V / x^T tiles.
    This lets the pair-0 Q projection start ~3us in, chasing the x chunks.
  - The pair loop is software-pipelined at (jt, ih) unit granularity:
    each unit issues the scores matmuls for pair p, the ctx matmuls for
    pair p-1, and filler projection matmuls for pair p+1/V, so the ScalarE
    exp stream always overlaps dense PE work and the single scores PSUM
    tile never stalls the PE.
  - Softmax normalization: the ctx matmul is padded to M=80 (V' strip =
    [V_h | ones | zeros*15]; a matmul costs N cycles regardless of M), so
    the [80, S] PSUM tile (64 ctx rows + denominator row + zero pad) can
    be drained with one DVE copy and XBAR'd as a block. The reciprocal
    then runs in [s-partition, 8] layout (DVE is free-dim-serial; a
    [1,1024] reciprocal costs 6.5us, a [128,8] one ~200ns), and the
    normalize is 8 per-partition tensor_scalar muls.
"""

import numpy as np
from contextlib import ExitStack

import concourse.bass as bass
import concourse.mybir as mybir
import concourse.tile as tile
from concourse import bacc
from concourse.bass_utils import run_bass_kernel_spmd

B, S, H, D = 8, 1024, 16, 64
W = H * D  # 1024
P = 128
N_CORES = 8
F32 = mybir.dt.float32
BF16 = mybir.dt.bfloat16
AF = mybir.ActivationFunctionType
ALU = mybir.AluOpType

ST = S // P   # 8 s-tiles
KT_ = W // P  # 8 contraction tiles
IH = 2        # 512-wide halves of the moving dim
HDP = 80      # padded V' width per head: 64 V cols + ones col + 15 zeros
NP = H // 2   # 8 head pairs


def _dedup_ldweights(nc):
    """Drop InstLdweights that reload the exact weights already resident in
    the PE array (kt-outer projection loops share one stationary between the
    two ih-half matmuls). Runs post-compile, so syncs are final."""
    removed = 0
    for f in nc.m.functions:
        for blk in f.blocks:
            ins = blk.instructions
            last_key = None
            to_remove = []
            for i in ins:
                if str(getattr(i, "engine", None)) != "EngineType.PE":
                    continue
                tn = type(i).__name__
                if tn == "InstLdweights":
                    si = i.sync_info
                    clean = si is None or (not si.on_wait and not si.on_update)
                    key = (str(i.ins), str(getattr(i, "is_transpose", None)),
                           str(getattr(i, "tile_position", None)),
                           str(getattr(i, "perf_mode", None)))
                    if clean and key == last_key:
                        to_remove.append(i)
                    else:
                        last_key = key
                elif tn != "InstMatmult":
                    last_key = None
            for i in to_remove:
                ins.remove(i)
            removed += len(to_remove)
    return removed


def build_kernel():
    nc = bacc.Bacc(trn_type="TRN2", target_bir_lowering=False, debug=False,
                   num_devices=N_CORES)

    xf_ext = nc.dram_tensor("from_tensor", [S, W], F32, kind="ExternalInput").ap()
    xt_ext = nc.dram_tensor("to_tensor", [S, W], F32, kind="ExternalInput").ap()
    wq_ext = nc.dram_tensor("Wq", [W, W], F32, kind="ExternalInput").ap()
    bq_ext = nc.dram_tensor("bq", [W], F32, kind="ExternalInput").ap()
    wk_ext = nc.dram_tensor("Wk", [W, W], F32, kind="ExternalInput").ap()
    bk_ext = nc.dram_tensor("bk", [W], F32, kind="ExternalInput").ap()
    wv_ext = nc.dram_tensor("Wv", [W, W], F32, kind="ExternalInput").ap()
    bv_ext = nc.dram_tensor("bv", [W], F32, kind="ExternalInput").ap()
    out_ext = nc.dram_tensor("out", [S, W], F32, kind="ExternalOutput").ap()

    with tile.TileContext(nc) as tc, ExitStack() as top:
        const = top.enter_context(tc.tile_pool(name="const", bufs=1))
        big = top.enter_context(tc.tile_pool(name="big", bufs=1))

        # per-pair per-partition bias scalars for Q^T/K^T (m on partitions)
        bq_sb = const.tile([P, KT_], F32, tag="bq")
        bk_sb = const.tile([P, KT_], F32, tag="bk")
        bv_row = const.tile([1, W], F32, tag="bvr")
        bvb = const.tile([P, W], F32, tag="bvb")

        # x^T split into two s-halves (each the product of 4 XBARs):
        # xT_h[i][p, kt*512 + s] = x[i*512 + s, kt*128+p]
        xTf_h = [big.tile([P, KT_ * 512], BF16, tag=f"xTf{i}", name=f"xTf{i}")
                 for i in range(IH)]
        xTt_h = [big.tile([P, KT_ * 512], BF16, tag=f"xTt{i}", name=f"xTt{i}")
                 for i in range(IH)]
        # per-pair weight slices: wq_t[p][pp, kt*128 + c] = Wq[kt*128+pp, p*128+c]
        wq_t = [big.tile([P, KT_ * P], BF16, tag=f"wq{p}", name=f"wq{p}")
                for p in range(NP)]
        wk_t = [big.tile([P, KT_ * P], BF16, tag=f"wk{p}", name=f"wk{p}")
                for p in range(NP)]
        # Wv column halves (heads 0-7 / 8-15)
        wv_t = [big.tile([P, KT_ * 512], BF16, tag=f"wv{i}", name=f"wv{i}")
                for i in range(2)]
        # V in [j, m] layout, padded strips:
        # V_t[half][j, (st*8 + hl)*80 + c]: c 0-63 = V, 64 = 1.0, 65-79 = 0
        V_t = [big.tile([P, ST * 8 * HDP], BF16, tag=f"vt{i}", name=f"vt{i}")
               for i in range(2)]

        def load_w_slice(dst, src, c0, c1):
            nc.gpsimd.dma_start(
                dst.rearrange("p (t f) -> p t f", f=c1 - c0),
                src.rearrange("(t p) f -> p t f", p=P)[:, :, c0:c1])

        with ExitStack() as stg_ctx:
            stg = stg_ctx.enter_context(tc.tile_pool(name="stg", bufs=3))

            def stage_x(x_ext, xT_h):
                """Load x in 1MB chunks (cast to bf16) + XBAR-transpose."""
                for c in range(4):
                    xs = stg.tile([P, 2 * W], BF16, tag="xs", name=f"xs{c}")
                    nc.gpsimd.dma_start(
                        xs.rearrange("p (t w) -> p t w", w=W),
                        x_ext.rearrange("(t p) w -> p t w", p=P)[
                            :, 2 * c:2 * c + 2, :])
                    for t in range(2):
                        ch = 2 * c + t  # global 128-row chunk index
                        # out[wp, wt, s] = xs_t[s, wt*128+wp]
                        nc.sync.dma_start(
                            xT_h[ch // 4].rearrange("p (t s) -> p t s", s=512)[
                                :, :, (ch % 4) * P:(ch % 4 + 1) * P],
                            xs[:, t * W:(t + 1) * W], transpose=True)

            # ---- DMA schedule (gpsimd/SWDGE queue, casts f32->bf16) ----
            load_w_slice(wq_t[0], wq_ext, 0, P)
            stage_x(xf_ext, xTf_h)
            nc.gpsimd.dma_start(bq_sb[:], bq_ext.rearrange("(t p) -> p t", p=P))
            nc.gpsimd.dma_start(bk_sb[:], bk_ext.rearrange("(t p) -> p t", p=P))
            nc.gpsimd.dma_start(bv_row[:],
                                bv_ext.rearrange("(p w) -> p w", p=1))
            nc.gpsimd.partition_broadcast(bvb[:], bv_row[:])
            load_w_slice(wk_t[0], wk_ext, 0, P)
            stage_x(xt_ext, xTt_h)
            load_w_slice(wv_t[0], wv_ext, 0, 512)      # V half A (heads 0-7)
            load_w_slice(wq_t[1], wq_ext, P, 2 * P)
            load_w_slice(wk_t[1], wk_ext, P, 2 * P)
            load_w_slice(wq_t[2], wq_ext, 2 * P, 3 * P)
            load_w_slice(wk_t[2], wk_ext, 2 * P, 3 * P)
            load_w_slice(wv_t[1], wv_ext, 512, 1024)   # V half B (heads 8-15)
            for mt in range(3, NP):
                load_w_slice(wq_t[mt], wq_ext, mt * P, (mt + 1) * P)
                load_w_slice(wk_t[mt], wk_ext, mt * P, (mt + 1) * P)

            # ---- pair loop ----
            with ExitStack() as ph2:
                pp_pool = ph2.enter_context(tc.tile_pool(name="pp", bufs=1))
                et_pool = ph2.enter_context(tc.tile_pool(name="et", bufs=20))
                sm_pool = ph2.enter_context(tc.tile_pool(name="sm", bufs=1))
                ps_proj = ph2.enter_context(
                    tc.tile_pool(name="ps_proj", bufs=2, space="PSUM"))
                ps_s = ph2.enter_context(
                    tc.tile_pool(name="ps_s", bufs=1, space="PSUM"))
                ps_c = ph2.enter_context(
                    tc.tile_pool(name="ps_c", bufs=2, space="PSUM"))

                def gen_qk_proj(QTp, KTp, mt):
                    """Q^T/K^T projection for pair mt, kt-outer (the two
                    ih-half matmuls share one ldweights after dedup)."""
                    for (dstT, w_t, x_h, b_sb) in (
                            (QTp, wq_t[mt], xTf_h, bq_sb),
                            (KTp, wk_t[mt], xTt_h, bk_sb)):
                        ps = {}
                        for ih in range(IH):
                            ps[ih] = ps_proj.tile([P, 512], F32, tag="proj",
                                                  name="pp")
                        for kt in range(KT_):
                            for ih in range(IH):
                                nc.tensor.matmul(
                                    ps[ih][:],
                                    lhsT=w_t[:, kt * P:(kt + 1) * P],
                                    rhs=x_h[ih][:, kt * 512:(kt + 1) * 512],
                                    start=(kt == 0), stop=(kt == KT_ - 1))
                            if kt % 4 == 3:
                                yield
                        for ih in range(IH):
                            nc.vector.tensor_scalar_add(
                                dstT[:, ih * 512:(ih + 1) * 512], ps[ih][:],
                                b_sb[:, mt:mt + 1])
                        yield

                def gen_v_proj(half):
                    """V projection for one 512-wide column half (8 heads),
                    all 8 s-tiles; yields after each s-tile."""
                    for st in range(ST):
                        pv = ps_proj.tile([P, 512], F32, tag="proj", name="pv")
                        for kt in range(KT_):
                            nc.tensor.matmul(
                                pv[:],
                                lhsT=xTt_h[st // 4][
                                    :, kt * 512 + (st % 4) * P:
                                    kt * 512 + (st % 4 + 1) * P],
                                rhs=wv_t[half][:, kt * 512:(kt + 1) * 512],
                                start=(kt == 0), stop=(kt == KT_ - 1))
                        dst = V_t[half][:, st * 8 * HDP:(st + 1) * 8 * HDP
                                        ].rearrange("p (h c) -> p h c", c=HDP)
                        nc.vector.tensor_tensor(
                            dst[:, :, 0:D],
                            pv[:].rearrange("p (h c) -> p h c", c=D),
                            bvb[:, half * 512:(half + 1) * 512].rearrange(
                                "p (h c) -> p h c", c=D),
                            ALU.add)
                        nc.vector.memset(dst[:, :, D:D + 1], 1.0)
                        nc.vector.memset(dst[:, :, D + 1:HDP], 0.0)
                        yield

                filler = []      # FIFO of generators of PE work chunks
                done_gens = set()

                def pull_filler():
                    while filler:
                        try:
                            next(filler[0])
                            return
                        except StopIteration:
                            done_gens.add(id(filler[0]))
                            filler.pop(0)

                def drain_gen(g):
                    if g is None or id(g) in done_gens:
                        return
                    while True:
                        try:
                            next(g)
                        except StopIteration:
                            done_gens.add(id(g))
                            if g in filler:
                                filler.remove(g)
                            return

                QK = {}      # pair -> (QTp, KTp)
                qk_gen = {}  # pair -> generator
                v_gen = {}   # half -> generator

                def emit_pair_qk(p):
                    QTp = pp_pool.tile([P, S], BF16, tag="qt", bufs=2,
                                       name="QTp")
                    KTp = pp_pool.tile([P, S], BF16, tag="kt", bufs=2,
                                       name="KTp")
                    QK[p] = (QTp, KTp)
                    g = gen_qk_proj(QTp, KTp, p)
                    qk_gen[p] = g
                    filler.append(g)

                Et = {}  # (pair, jt, ih) -> exp tile
                pc = {}  # (pair, hh) -> ctx psum tile

                def emit_scores_unit(p, jt, ih):
                    QTp, KTp = QK[p]
                    pss = ps_s.tile([P, 1024], F32, tag="pss", name="pss")
                    for hh in range(2):
                        ho = hh * D
                        nc.tensor.matmul(
                            pss[:, hh * 512:(hh + 1) * 512],
                            lhsT=KTp[ho:ho + D, jt * P: jt * P + P],
                            rhs=QTp[ho:ho + D, ih * 512:(ih + 1) * 512],
                            start=True, stop=True)
                    et = et_pool.tile([P, 1024], BF16, tag="et", name="et")
                    nc.scalar.activation(et[:], pss[:], AF.Exp, scale=0.125)
                    Et[(p, jt, ih)] = et

                def emit_ctx_unit(p, jt, ih):
                    if jt == 0 and ih == 0:
                        for hh in range(2):
                            pc[(p, hh)] = ps_c.tile([HDP, S], F32, tag="pcc",
                                                    name="pcc")
                    et = Et.pop((p, jt, ih))
                    half, hb = p // 4, (p % 4) * 2
                    for hh in range(2):
                        nc.tensor.matmul(
                            pc[(p, hh)][:, ih * 512:(ih + 1) * 512],
                            lhsT=V_t[half][:, (jt * 8 + hb + hh) * HDP:
                                           (jt * 8 + hb + hh + 1) * HDP],
                            rhs=et[:, hh * 512:(hh + 1) * 512],
                            start=(jt == 0), stop=(jt == ST - 1))

                def emit_out(p):
                    """Drain + XBAR-transpose + normalize + DMA for pair p."""
                    out_p = sm_pool.tile([P, ST * P], BF16, tag="outp", bufs=2,
                                         name="out_p")
                    for hh in range(2):
                        pch = pc.pop((p, hh))
                        cb = sm_pool.tile([HDP, S], BF16, tag="cb", bufs=3,
                                          name="cb")
                        nc.vector.tensor_copy(cb[:], pch[:])
                        # out_u[s, it, c] = cb[c, it*128+s]
                        out_u = sm_pool.tile([P, ST * HDP], BF16, tag="outu",
                                             bufs=3, name="out_u")
                        nc.sync.dma_start(
                            out_u.rearrange("p (t c) -> p t c", c=HDP),
                            cb[:], transpose=True)
                        rinv = sm_pool.tile([P, ST], F32, tag="rinv", bufs=3,
                                            name="rinv")
                        nc.vector.reciprocal(
                            rinv[:],
                            out_u.rearrange("p (t c) -> p t c", c=HDP)[
                                :, :, D:D + 1])
                        for it in range(ST):
                            nc.vector.tensor_scalar_mul(
                                out_p[:, it * P + hh * D:
                                      it * P + hh * D + D],
                                out_u[:, it * HDP: it * HDP + D],
                                rinv[:, it:it + 1])
                    nc.gpsimd.dma_start(
                        out_ext.rearrange("(t p) (g c) -> p t g c", p=P, c=P)[
                            :, :, p, :],
                        out_p.rearrange("p (t c) -> p t c", c=P))

                # ---- pipeline ----
                emit_pair_qk(0)
                drain_gen(qk_gen[0])  # pair 0's Q/K must fully precede units
                emit_pair_qk(1)
                v_gen[0] = gen_v_proj(0)
                filler.append(v_gen[0])

                for p in range(NP):
                    if p == 1:
                        v_gen[1] = gen_v_proj(1)
                        filler.append(v_gen[1])
                    if p + 2 < NP:
                        emit_pair_qk(p + 2)
                    # safety: everything pair p's scores / pair p-1's ctx
                    # read must already be in the PE queue
                    drain_gen(qk_gen.get(p))
                    if p > 0:
                        drain_gen(v_gen.get((p - 1) // 4))
                    for jt in range(ST):
                        for ih in range(IH):
                            emit_scores_unit(p, jt, ih)
                            if p > 0:
                                emit_ctx_unit(p - 1, jt, ih)
                            pull_filler()
                            if p == 0:
                                pull_filler()
                    if p > 0:
                        emit_out(p - 1)
                # drain pair NP-1's ctx + out
                for jt in range(ST):
                    for ih in range(IH):
                        emit_ctx_unit(NP - 1, jt, ih)
                        pull_filler()
                emit_out(NP - 1)

    nc.compile()
    _dedup_ldweights(nc)
    return nc


def run(inputs, trace=False, trace_kwargs=None):
    """inputs: dict of full-shape np arrays as in reference.setup_inputs()."""
    nc = build_kernel()
    in_maps = []
    for b in range(N_CORES):
        in_maps.append({
            "from_tensor": np.ascontiguousarray(np.asarray(inputs["from_tensor"][b], dtype=np.float32)),
            "to_tensor": np.ascontiguousarray(np.asarray(inputs["to_tensor"][b], dtype=np.float32)),
            "Wq": np.asarray(inputs["Wq"], dtype=np.float32),
            "bq": np.asarray(inputs["bq"], dtype=np.float32),
            "Wk": np.asarray(inputs["Wk"], dtype=np.float32),
            "bk": np.asarray(inputs["bk"], dtype=np.float32),
            "Wv": np.asarray(inputs["Wv"], dtype=np.float32),
            "bv": np.asarray(inputs["bv"], dtype=np.float32),
        })
    res = run_bass_kernel_spmd(nc, in_maps, core_ids=list(range(N_CORES)),
                               trace=trace, **(trace_kwargs or {}))
    out = np.stack([np.asarray(res.results[b]["out"]) for b in range(N_CORES)],
                   axis=0).astype(np.float32)
    return out, res


def kernel(**inputs):
    out, _ = run(inputs, trace=False)
    return out


# revision 9
# speedup vs baseline: 1.3587x; 1.0060x over previous
"""Multi-head attention forward (B=8, S=1024, H=16, D=64) on 8 TRN2 NeuronCores.

Sharding: pure data-parallel over batch - core b computes batch element b
end-to-end (QKV projections + 16-head attention). Zero collectives.

v3 design notes:
  - ZERO PE transposes. x^T comes from XBAR DMA-transposes of bf16 staging
    chunks (cast f32->bf16 on the SWDGE load). V is computed directly in
    [seq, dim] layout, and the final ctx^T -> [seq, dim] flip is another
    XBAR DMA-transpose.
  - Tile dependencies are whole-tile, so every independently-DMA'd slice
    gets its own tile: per-pair Wq/Wk tiles, per-half Wv / V / x^T tiles.
    This lets the pair-0 Q projection start ~3us in, chasing the x chunks.
  - The pair loop is software-pipelined at (jt, ih) unit granularity:
    each unit issues the scores matmuls for pair p, the ctx matmuls for
    pair p-1, and filler projection matmuls for pair p+1/V, so the ScalarE
    exp stream always overlaps dense PE work and the single scores PSUM
    tile never stalls the PE.
  - Softmax normalization: the ctx matmul is padded to M=80 (V' strip =
    [V_h | ones | zeros*15]; a matmul costs N cycles regardless of M), so
    the [80, S] PSUM tile (64 ctx rows + denominator row + zero pad) can
    be drained with one DVE copy and XBAR'd as a block. The reciprocal
    then runs in [s-partition, 8] layout (DVE is free-dim-serial; a
    [1,1024] reciprocal costs 6.5us, a [128,8] one ~200ns), and the
    normalize is 8 per-partition tensor_scalar muls.
"""

import numpy as np
from contextlib import ExitStack

import concourse.bass as bass
import concourse.mybir as mybir
import concourse.tile as tile
from concourse import bacc
from concourse.bass_utils import run_bass_kernel_spmd

B, S, H, D = 8, 1024, 16, 64
W = H * D  # 1024
P = 128
N_CORES = 8
F32 = mybir.dt.float32
BF16 = mybir.dt.bfloat16
AF = mybir.ActivationFunctionType
ALU = mybir.AluOpType

ST = S // P   # 8 s-tiles
KT_ = W // P  # 8 contraction tiles
IH = 2        # 512-wide halves of the moving dim
HDP = 80      # padded V' width per head: 64 V cols + ones col + 15 zeros
NP = H // 2   # 8 head pairs


def _dedup_ldweights(nc):
    """Drop InstLdweights that reload the exact weights already resident in
    the PE array (kt-outer projection loops share one stationary between the
    two ih-half matmuls). Runs post-compile, so syncs are final."""
    removed = 0
    for f in nc.m.functions:
        for blk in f.blocks:
            ins = blk.instructions
            last_key = None
            to_remove = []
            for i in ins:
                if str(getattr(i, "engine", None)) != "EngineType.PE":
                    continue
                tn = type(i).__name__
                if tn == "InstLdweights":
                    si = i.sync_info
                    clean = si is None or (not si.on_wait and not si.on_update)
                    key = (str(i.ins), str(getattr(i, "is_transpose", None)),
                           str(getattr(i, "tile_position", None)),
                           str(getattr(i, "perf_mode", None)))
                    if clean and key == last_key:
                        to_remove.append(i)
                    else:
                        last_key = key
                elif tn != "InstMatmult":
                    last_key = None
            for i in to_remove:
                ins.remove(i)
            removed += len(to_remove)
    return removed


def build_kernel():
    nc = bacc.Bacc(trn_type="TRN2", target_bir_lowering=False, debug=False,
                   num_devices=N_CORES)

    xf_ext = nc.dram_tensor("from_tensor", [S, W], F32, kind="ExternalInput").ap()
    xt_ext = nc.dram_tensor("to_tensor", [S, W], F32, kind="ExternalInput").ap()
    wq_ext = nc.dram_tensor("Wq", [W, W], F32, kind="ExternalInput").ap()
    bq_ext = nc.dram_tensor("bq", [W], F32, kind="ExternalInput").ap()
    wk_ext = nc.dram_tensor("Wk", [W, W], F32, kind="ExternalInput").ap()
    bk_ext = nc.dram_tensor("bk", [W], F32, kind="ExternalInput").ap()
    wv_ext = nc.dram_tensor("Wv", [W, W], F32, kind="ExternalInput").ap()
    bv_ext = nc.dram_tensor("bv", [W], F32, kind="ExternalInput").ap()
    out_ext = nc.dram_tensor("out", [S, W], F32, kind="ExternalOutput").ap()

    with tile.TileContext(nc) as tc, ExitStack() as top:
        const = top.enter_context(tc.tile_pool(name="const", bufs=1))
        big = top.enter_context(tc.tile_pool(name="big", bufs=1))

        # per-pair per-partition bias scalars for Q^T/K^T (m on partitions)
        bq_sb = const.tile([P, KT_], F32, tag="bq")
        bk_sb = const.tile([P, KT_], F32, tag="bk")
        bv_row = const.tile([1, W], F32, tag="bvr")
        bvb = const.tile([P, W], F32, tag="bvb")

        # x^T split into two s-halves (each the product of 4 XBARs):
        # xT_h[i][p, kt*512 + s] = x[i*512 + s, kt*128+p]
        xTf_h = [big.tile([P, KT_ * 512], BF16, tag=f"xTf{i}", name=f"xTf{i}")
                 for i in range(IH)]
        xTt_h = [big.tile([P, KT_ * 512], BF16, tag=f"xTt{i}", name=f"xTt{i}")
                 for i in range(IH)]
        # weight column quarters (2 pairs each; 512B bf16 DMA rows keep the
        # SWDGE convert path at line rate - 256B rows would halve it):
        # wq_q[i][pp, kt*256 + c] = Wq[kt*128+pp, i*256+c]
        wq_q = [big.tile([P, KT_ * 256], BF16, tag=f"wq{i}", name=f"wq{i}")
                for i in range(4)]
        wk_q = [big.tile([P, KT_ * 256], BF16, tag=f"wk{i}", name=f"wk{i}")
                for i in range(4)]
        # Wv column halves (heads 0-7 / 8-15)
        wv_t = [big.tile([P, KT_ * 512], BF16, tag=f"wv{i}", name=f"wv{i}")
                for i in range(2)]
        # V in [j, m] layout, padded strips:
        # V_t[half][j, (st*8 + hl)*80 + c]: c 0-63 = V, 64 = 1.0, 65-79 = 0
        V_t = [big.tile([P, ST * 8 * HDP], BF16, tag=f"vt{i}", name=f"vt{i}")
               for i in range(2)]

        def load_w_slice(dst, src, c0, c1):
            nc.gpsimd.dma_start(
                dst.rearrange("p (t f) -> p t f", f=c1 - c0),
                src.rearrange("(t p) f -> p t f", p=P)[:, :, c0:c1])

        with ExitStack() as stg_ctx:
            stg = stg_ctx.enter_context(tc.tile_pool(name="stg", bufs=3))

            def stage_x(x_ext, xT_h):
                """Load x in 1MB chunks (cast to bf16) + XBAR-transpose."""
                for c in range(4):
                    xs = stg.tile([P, 2 * W], BF16, tag="xs", name=f"xs{c}")
                    nc.gpsimd.dma_start(
                        xs.rearrange("p (t w) -> p t w", w=W),
                        x_ext.rearrange("(t p) w -> p t w", p=P)[
                            :, 2 * c:2 * c + 2, :])
                    for t in range(2):
                        ch = 2 * c + t  # global 128-row chunk index
                        # out[wp, wt, s] = xs_t[s, wt*128+wp]
                        nc.sync.dma_start(
                            xT_h[ch // 4].rearrange("p (t s) -> p t s", s=512)[
                                :, :, (ch % 4) * P:(ch % 4 + 1) * P],
                            xs[:, t * W:(t + 1) * W], transpose=True)

            # ---- DMA schedule (gpsimd/SWDGE queue, casts f32->bf16) ----
            load_w_slice(wq_q[0], wq_ext, 0, 256)      # pairs 0-1 Q cols
            stage_x(xf_ext, xTf_h)
            nc.gpsimd.dma_start(bq_sb[:], bq_ext.rearrange("(t p) -> p t", p=P))
            nc.gpsimd.dma_start(bk_sb[:], bk_ext.rearrange("(t p) -> p t", p=P))
            nc.gpsimd.dma_start(bv_row[:],
                                bv_ext.rearrange("(p w) -> p w", p=1))
            nc.gpsimd.partition_broadcast(bvb[:], bv_row[:])
            load_w_slice(wk_q[0], wk_ext, 0, 256)      # pairs 0-1 K cols
            stage_x(xt_ext, xTt_h)
            load_w_slice(wv_t[0], wv_ext, 0, 512)      # V half A (heads 0-7)
            load_w_slice(wq_q[1], wq_ext, 256, 512)
            load_w_slice(wk_q[1], wk_ext, 256, 512)
            load_w_slice(wq_q[2], wq_ext, 512, 768)
            load_w_slice(wk_q[2], wk_ext, 512, 768)
            load_w_slice(wv_t[1], wv_ext, 512, 1024)   # V half B (heads 8-15)
            load_w_slice(wq_q[3], wq_ext, 768, 1024)
            load_w_slice(wk_q[3], wk_ext, 768, 1024)

            # ---- pair loop ----
            with ExitStack() as ph2:
                pp_pool = ph2.enter_context(tc.tile_pool(name="pp", bufs=1))
                et_pool = ph2.enter_context(tc.tile_pool(name="et", bufs=20))
                sm_pool = ph2.enter_context(tc.tile_pool(name="sm", bufs=1))
                ps_proj = ph2.enter_context(
                    tc.tile_pool(name="ps_proj", bufs=2, space="PSUM"))
                ps_s = ph2.enter_context(
                    tc.tile_pool(name="ps_s", bufs=1, space="PSUM"))
                ps_c = ph2.enter_context(
                    tc.tile_pool(name="ps_c", bufs=2, space="PSUM"))

                def gen_qk_proj(QTp, KTp, mt):
                    """Q^T/K^T projection for pair mt, kt-outer (the two
                    ih-half matmuls share one ldweights after dedup)."""
                    for (dstT, w_t, x_h, b_sb) in (
                            (QTp, wq_q[mt // 2], xTf_h, bq_sb),
                            (KTp, wk_q[mt // 2], xTt_h, bk_sb)):
                        wo = (mt % 2) * P
                        ps = {}
                        for ih in range(IH):
                            ps[ih] = ps_proj.tile([P, 512], F32, tag="proj",
                                                  name="pp")
                        for kt in range(KT_):
                            for ih in range(IH):
                                nc.tensor.matmul(
                                    ps[ih][:],
                                    lhsT=w_t[:, kt * 256 + wo:
                                             kt * 256 + wo + P],
                                    rhs=x_h[ih][:, kt * 512:(kt + 1) * 512],
                                    start=(kt == 0), stop=(kt == KT_ - 1))
                            if kt % 4 == 3:
                                yield
                        for ih in range(IH):
                            nc.vector.tensor_scalar_add(
                                dstT[:, ih * 512:(ih + 1) * 512], ps[ih][:],
                                b_sb[:, mt:mt + 1])
                        yield

                def gen_v_proj(half):
                    """V projection for one 512-wide column half (8 heads),
                    all 8 s-tiles; yields after each s-tile."""
                    for st in range(ST):
                        pv = ps_proj.tile([P, 512], F32, tag="proj", name="pv")
                        for kt in range(KT_):
                            nc.tensor.matmul(
                                pv[:],
                                lhsT=xTt_h[st // 4][
                                    :, kt * 512 + (st % 4) * P:
                                    kt * 512 + (st % 4 + 1) * P],
                                rhs=wv_t[half][:, kt * 512:(kt + 1) * 512],
                                start=(kt == 0), stop=(kt == KT_ - 1))
                        dst = V_t[half][:, st * 8 * HDP:(st + 1) * 8 * HDP
                                        ].rearrange("p (h c) -> p h c", c=HDP)
                        nc.vector.tensor_tensor(
                            dst[:, :, 0:D],
                            pv[:].rearrange("p (h c) -> p h c", c=D),
                            bvb[:, half * 512:(half + 1) * 512].rearrange(
                                "p (h c) -> p h c", c=D),
                            ALU.add)
                        nc.vector.memset(dst[:, :, D:D + 1], 1.0)
                        nc.vector.memset(dst[:, :, D + 1:HDP], 0.0)
                        yield

                filler = []      # FIFO of generators of PE work chunks
                done_gens = set()

                def pull_filler():
                    while filler:
                        try:
                            next(filler[0])
                            return
                        except StopIteration:
                            done_gens.add(id(filler[0]))
                            filler.pop(0)

                def drain_gen(g):
                    if g is None or id(g) in done_gens:
                        return
                    while True:
                        try:
                            next(g)
                        except StopIteration:
                            done_gens.add(id(g))
                            if g in filler:
                                filler.remove(g)
                            return

                QK = {}      # pair -> (QTp, KTp)
                qk_gen = {}  # pair -> generator
                v_gen = {}   # half -> generator

                def emit_pair_qk(p):
                    QTp = pp_pool.tile([P, S], BF16, tag="qt", bufs=2,
                                       name="QTp")
                    KTp = pp_pool.tile([P, S], BF16, tag="kt", bufs=2,
                                       name="KTp")
                    QK[p] = (QTp, KTp)
                    g = gen_qk_proj(QTp, KTp, p)
                    qk_gen[p] = g
                    filler.append(g)

                Et = {}  # (pair, jt, ih) -> exp tile
                pc = {}  # (pair, hh) -> ctx psum tile

                def emit_scores_unit(p, jt, ih):
                    QTp, KTp = QK[p]
                    pss = ps_s.tile([P, 1024], F32, tag="pss", name="pss")
                    for hh in range(2):
                        ho = hh * D
                        nc.tensor.matmul(
                            pss[:, hh * 512:(hh + 1) * 512],
                            lhsT=KTp[ho:ho + D, jt * P: jt * P + P],
                            rhs=QTp[ho:ho + D, ih * 512:(ih + 1) * 512],
                            start=True, stop=True)
                    et = et_pool.tile([P, 1024], BF16, tag="et", name="et")
                    nc.scalar.activation(et[:], pss[:], AF.Exp, scale=0.125)
                    Et[(p, jt, ih)] = et

                def emit_ctx_unit(p, jt, ih):
                    if jt == 0 and ih == 0:
                        for hh in range(2):
                            pc[(p, hh)] = ps_c.tile([HDP, S], F32, tag="pcc",
                                                    name="pcc")
                    et = Et.pop((p, jt, ih))
                    half, hb = p // 4, (p % 4) * 2
                    for hh in range(2):
                        nc.tensor.matmul(
                            pc[(p, hh)][:, ih * 512:(ih + 1) * 512],
                            lhsT=V_t[half][:, (jt * 8 + hb + hh) * HDP:
                                           (jt * 8 + hb + hh + 1) * HDP],
                            rhs=et[:, hh * 512:(hh + 1) * 512],
                            start=(jt == 0), stop=(jt == ST - 1))

                def emit_out(p):
                    """Drain + XBAR-transpose + normalize + DMA for pair p."""
                    out_p = sm_pool.tile([P, ST * P], BF16, tag="outp", bufs=2,
                                         name="out_p")
                    for hh in range(2):
                        pch = pc.pop((p, hh))
                        cb = sm_pool.tile([HDP, S], BF16, tag="cb", bufs=3,
                                          name="cb")
                        nc.vector.tensor_copy(cb[:], pch[:])
                        # out_u[s, it, c] = cb[c, it*128+s]
                        out_u = sm_pool.tile([P, ST * HDP], BF16, tag="outu",
                                             bufs=3, name="out_u")
                        nc.sync.dma_start(
                            out_u.rearrange("p (t c) -> p t c", c=HDP),
                            cb[:], transpose=True)
                        rinv = sm_pool.tile([P, ST], F32, tag="rinv", bufs=3,
                                            name="rinv")
                        nc.vector.reciprocal(
                            rinv[:],
                            out_u.rearrange("p (t c) -> p t c", c=HDP)[
                                :, :, D:D + 1])
                        for it in range(ST):
                            nc.vector.tensor_scalar_mul(
                                out_p[:, it * P + hh * D:
                                      it * P + hh * D + D],
                                out_u[:, it * HDP: it * HDP + D],
                                rinv[:, it:it + 1])
                    nc.gpsimd.dma_start(
                        out_ext.rearrange("(t p) (g c) -> p t g c", p=P, c=P)[
                            :, :, p, :],
                        out_p.rearrange("p (t c) -> p t c", c=P))

                # ---- pipeline ----
                emit_pair_qk(0)
                drain_gen(qk_gen[0])  # pair 0's Q/K must fully precede units
                emit_pair_qk(1)
                v_gen[0] = gen_v_proj(0)
                filler.append(v_gen[0])

                for p in range(NP):
                    if p == 1:
                        v_gen[1] = gen_v_proj(1)
                        filler.append(v_gen[1])
                    if p + 2 < NP:
                        emit_pair_qk(p + 2)
                    # safety: everything pair p's scores / pair p-1's ctx
                    # read must already be in the PE queue
                    drain_gen(qk_gen.get(p))
                    if p > 0:
                        drain_gen(v_gen.get((p - 1) // 4))
                    for jt in range(ST):
                        for ih in range(IH):
                            emit_scores_unit(p, jt, ih)
                            if p > 0:
                                emit_ctx_unit(p - 1, jt, ih)
                            pull_filler()
                            if p == 0:
                                pull_filler()
                    if p > 0:
                        emit_out(p - 1)
                # drain pair NP-1's ctx + out
                for jt in range(ST):
                    for ih in range(IH):
                        emit_ctx_unit(NP - 1, jt, ih)
                        pull_filler()
                emit_out(NP - 1)

    nc.compile()
    _dedup_ldweights(nc)
    return nc


def run(inputs, trace=False, trace_kwargs=None):
    """inputs: dict of full-shape np arrays as in reference.setup_inputs()."""
    nc = build_kernel()
    in_maps = []
    for b in range(N_CORES):
        in_maps.append({
            "from_tensor": np.ascontiguousarray(np.asarray(inputs["from_tensor"][b], dtype=np.float32)),
            "to_tensor": np.ascontiguousarray(np.asarray(inputs["to_tensor"][b], dtype=np.float32)),
            "Wq": np.asarray(inputs["Wq"], dtype=np.float32),
            "bq": np.asarray(inputs["bq"], dtype=np.float32),
            "Wk": np.asarray(inputs["Wk"], dtype=np.float32),
            "bk": np.asarray(inputs["bk"], dtype=np.float32),
            "Wv": np.asarray(inputs["Wv"], dtype=np.float32),
            "bv": np.asarray(inputs["bv"], dtype=np.float32),
        })
    res = run_bass_kernel_spmd(nc, in_maps, core_ids=list(range(N_CORES)),
                               trace=trace, **(trace_kwargs or {}))
    out = np.stack([np.asarray(res.results[b]["out"]) for b in range(N_CORES)],
                   axis=0).astype(np.float32)
    return out, res


def kernel(**inputs):
    out, _ = run(inputs, trace=False)
    return out
